# revision 4
# baseline (speedup 1.0000x reference)
"""Trainium2 Bass kernel: CustomPatchEmbedding.

gather 16x16x3 patches at runtime (h_idx, w_idx) + 768x768 linear projection.

kernel(**inputs) takes FULL unsharded inputs
  x [32,3,384,384] f32, h_idx/w_idx [32,576] i32, proj_w [768,768] f32,
  proj_b [768] f32  ->  out [32,576,768] f32.

Sharding: data-parallel batch across 8 NeuronCores (4 images each).

Layout: the SWDGE indirect DMA supports ONE dynamic offset per partition per
instruction, streaming the dest free dim contiguously. To cover a 16-row
patch with only two such runs and ZERO garbage columns, the host packs an
8-fold row-replicated HWC bf16 tensor
    QR[b][hb][w][c][r] = x[b, c, hb+r, w]   (r = 0..7)
so the run at (b, hb, w) carries rows hb..hb+7 for 16 px x 3 ch = 384
elems. A patch at (h, w) is two runs (hb=h and hb=h+8): K = 768 exactly,
k = g*384 + dw*24 + c*8 + r. No weight variants, no dummy slots -> exactly
18 chunks of 128 patches, 2 gather instructions each.

The gathered G [patch, k] is transposed to [k, patch] with XBAR DMA
transposes on the SP/Activation HWDGE queues (not the tensor engine), then
6 k-block matmuls against a host-reordered weight W'[k, e] accumulate in
PSUM. DVE adds the bias and narrows to bf16; the result is stored bf16 and
upcast to f32 on the host.
"""

import numpy as np
import ml_dtypes

PH, PW = 16, 16
EMBED = 768
B, C, H, W = 32, 3, 384, 384
N = 576
NCORES = 8
BPC = B // NCORES            # images per core (4)
M = BPC * N                  # patches per core (2304)
NCHUNK = M // 128            # 18
K = C * PH * PW              # 768 contraction
NKB = K // 128               # 6 k-blocks
REP = 16                     # row replication factor of the packed QR
RUN = PW * C * REP           # 768 elems per gather run = one whole patch
V = BPC * H * W * C * REP    # elements in the core's QR slice
QCOLS = 256                  # q dram innermost dim (512 B in bf16)

_cache = {}


def _emit_body(nc, tc, bass, mybir, aps):
    dt = mybir.dt
    q_d, offs_d, w_d, bias_d, out_d = (
        aps["q"], aps["offs"], aps["wk"], aps["bias"], aps["out"])

    SC = 3
    NS = NCHUNK // SC            # 6 pipeline steps
    with tc.tile_pool(name="const", bufs=1) as cpool, \
         tc.tile_pool(name="gath", bufs=6) as gpool, \
         tc.tile_pool(name="gt", bufs=6) as tpool, \
         tc.tile_pool(name="psumt", bufs=4, space="PSUM") as apool, \
         tc.tile_pool(name="outp", bufs=4) as opool:
        # Emission order drives both per-engine program order and the global
        # HWDGE/SWDGE sem-lane rotation. Front-load the fill-critical chain
        # (offs -> g0 -> T0) and the k0 weight block; the remaining weight
        # blocks + bias dispatch while chunk 0's k0 matmuls run.
        offs_sb = cpool.tile([128, NCHUNK], dt.int32)
        nc.gpsimd.dma_start(out=offs_sb[:], in_=offs_d[:, :])
        w_sb = cpool.tile([128, NKB * EMBED], dt.bfloat16)
        nc.scalar.dma_start(out=w_sb[:, 0:EMBED], in_=w_d[0:128, :])

        G = [gpool.tile([128, SC * K], dt.bfloat16, tag="G",
                        name=f"G_{s}") for s in range(NS)]
        Gt = [tpool.tile([128, SC * NKB, 128], dt.bfloat16, tag="Gt",
                         name=f"Gt_{s}") for s in range(NS)]

        def gather(s):
            # per-chunk SWDGE gathers (walrus applies one offset per
            # partition per instruction; multi-window offset APs don't work)
            for u in range(SC):
                c = SC * s + u
                nc.gpsimd.indirect_dma_start(
                    out=G[s][:, u * RUN:(u + 1) * RUN],
                    out_offset=None,
                    in_=q_d[:, :],
                    in_offset=bass.IndirectOffsetOnAxis(
                        ap=offs_sb[:, c:c + 1], axis=1),
                )

        def transpose(s):
            # [128 m, SC*768 k] -> [128 k_lo, SC*6 k_hi, 128 m] XBAR
            nc.sync.dma_start(out=Gt[s][:], in_=G[s][:], transpose=True)

        gather(0)
        transpose(0)
        for k in range(1, NKB):
            nc.scalar.dma_start(out=w_sb[:, k * EMBED:(k + 1) * EMBED],
                                in_=w_d[k * 128:(k + 1) * 128, :])
        bias_sb = cpool.tile([128, EMBED], dt.float32)
        nc.scalar.dma_start(out=bias_sb[:], in_=bias_d[:, :])
        # all remaining gathers emitted consecutively: SWDGE lane recycling
        # then only waits on earlier gathers/offs (all complete early), never
        # on transposes or stores
        for s in range(1, NS):
            gather(s)
        for s in range(1, NS):
            transpose(s)

        for s in range(NS):
            ob = opool.tile([128, SC, EMBED], dt.bfloat16, tag="ob")
            for u in range(SC):
                acc = apool.tile([128, EMBED], dt.float32, tag="acc")
                for k in range(NKB):
                    lhsT = Gt[s][:, u * NKB + k, :]
                    nc.tensor.matmul(
                        acc[:, 0:512], lhsT,
                        w_sb[:, k * EMBED:k * EMBED + 512],
                        start=(k == 0), stop=(k == NKB - 1))
                    nc.tensor.matmul(
                        acc[:, 512:EMBED], lhsT,
                        w_sb[:, k * EMBED + 512:(k + 1) * EMBED],
                        start=(k == 0), stop=(k == NKB - 1))
                nc.vector.tensor_add(out=ob[:, u, :], in0=acc[:],
                                     in1=bias_sb[:])
            nc.scalar.dma_start(out=out_d[s * 128:(s + 1) * 128, :],
                                in_=ob[:])


def _build(n_cores=NCORES):
    import concourse.bass as bass
    import concourse.bacc as bacc
    import concourse.tile as tile
    import concourse.mybir as mybir

    dt = mybir.dt
    nc = bacc.Bacc("TRN2", target_bir_lowering=False, debug=False,
                   num_devices=n_cores)
    aps = {
        "q": nc.dram_tensor("q", [V // QCOLS, QCOLS], dt.bfloat16,
                            kind="ExternalInput").ap(),
        "offs": nc.dram_tensor("offs", [128, NCHUNK], dt.int32,
                               kind="ExternalInput").ap(),
        "wk": nc.dram_tensor("wk", [K, EMBED], dt.bfloat16,
                             kind="ExternalInput").ap(),
        "bias": nc.dram_tensor("bias", [128, EMBED], dt.float32,
                               kind="ExternalInput").ap(),
        "out": nc.dram_tensor("out", [(NCHUNK // 3) * 128, 3 * EMBED],
                              dt.bfloat16, kind="ExternalOutput").ap(),
    }
    with tile.TileContext(nc) as tc:
        _emit_body(nc, tc, bass, mybir, aps)
    nc.compile()
    return nc


def _pack_q(x_slice):
    """[BPC, C, H, W] f32 -> 16-fold row-replicated HWC bf16.

    QR[b, hb, w, c, r] = x[b, c, hb+r, w], r = 0..REP-1 (row hb+r clamped),
    so one run at (b, h, w) is the entire 16x16x3 patch.
    """
    xt = x_slice.transpose(0, 2, 3, 1).astype(ml_dtypes.bfloat16)
    xtp = np.pad(xt, ((0, 0), (0, REP - 1), (0, 0), (0, 0)), mode="edge")
    sw = np.lib.stride_tricks.sliding_window_view(xtp, REP, axis=1)
    q = np.ascontiguousarray(sw)                          # [b, hb, w, c, r]
    return q.reshape(V // QCOLS, QCOLS)


def _offsets(hb, wb):
    """[BPC, N] h/w -> [128, NCHUNK] i32 offsets, column t for chunk t,
    row p = patch t*128+p."""
    h = hb.reshape(M).astype(np.int64)
    w = wb.reshape(M).astype(np.int64)
    b = np.arange(M) // N
    off = ((b * H + h) * W + w) * (C * REP)
    off = off.reshape(NCHUNK, 128).T
    return np.ascontiguousarray(off).astype(np.int32)


def _w_reorder(proj_w):
    """[E, f_torch] -> [k, E] bf16 with k = dw*48 + c*16 + r (ph = r)."""
    dw, c, r = np.meshgrid(np.arange(PW), np.arange(C), np.arange(REP),
                           indexing="ij")
    f = (c * (PH * PW) + r * PW + dw).reshape(-1)
    return np.ascontiguousarray(proj_w.T[f, :]).astype(ml_dtypes.bfloat16)


def _in_maps(x, h_idx, w_idx, proj_w, proj_b):
    wk = _w_reorder(np.asarray(proj_w, np.float32))
    bias = np.ascontiguousarray(
        np.broadcast_to(np.asarray(proj_b, np.float32), (128, EMBED)))
    maps = []
    for core in range(NCORES):
        xs = np.asarray(x[core * BPC:(core + 1) * BPC], np.float32)
        hb = np.asarray(h_idx[core * BPC:(core + 1) * BPC])
        wb = np.asarray(w_idx[core * BPC:(core + 1) * BPC])
        maps.append({"q": _pack_q(xs), "offs": _offsets(hb, wb),
                     "wk": wk, "bias": bias})
    return maps


def _make_runner(nc, n_cores):
    """Jit the prebuilt Bass module once; reuse across calls."""
    import jax
    from jax.sharding import Mesh, PartitionSpec
    from jax.experimental.shard_map import shard_map
    import concourse.mybir as mybir
    from concourse import bass2jax

    bass2jax.install_neuronx_cc_hook()
    in_names, out_names, out_avals, zero_outs = [], [], [], []
    partition_name = (nc.partition_id_tensor.name
                      if nc.partition_id_tensor else None)
    for alloc in nc.m.functions[0].allocations:
        if not isinstance(alloc, mybir.MemoryLocationSet):
            continue
        if not alloc.memorylocations:
            continue
        name = alloc.memorylocations[0].name
        if alloc.kind == "ExternalInput":
            if name != partition_name:
                in_names.append(name)
        elif alloc.kind == "ExternalOutput":
            out_names.append(name)
            shape = tuple(alloc.tensor_shape)
            dtype = mybir.dt.np(alloc.dtype)
            out_avals.append(jax.core.ShapedArray(shape, dtype))
            zero_outs.append(np.zeros(shape, dtype))
    n_params = len(in_names)
    n_outs = len(out_avals)
    all_in_names = list(in_names) + list(out_names)
    if partition_name is not None:
        all_in_names.append(partition_name)
    donate = tuple(range(n_params, n_params + n_outs))

    def _body(*args):
        operands = list(args)
        if partition_name is not None:
            operands.append(bass2jax.partition_id_tensor())
        outs = bass2jax._bass_exec_p.bind(
            *operands,
            out_avals=tuple(out_avals),
            in_names=tuple(all_in_names),
            out_names=tuple(out_names),
            lowering_input_output_aliases=(),
            sim_require_finite=True,
            sim_require_nnan=True,
            nc=nc,
        )
        return tuple(outs)

    devices = jax.devices()[:n_cores]
    mesh = Mesh(np.asarray(devices), ("core",))
    in_specs = (PartitionSpec("core"),) * (n_params + n_outs)
    out_specs = (PartitionSpec("core"),) * n_outs
    jitted = jax.jit(
        shard_map(_body, mesh=mesh, in_specs=in_specs, out_specs=out_specs,
                  check_rep=False),
        donate_argnums=donate, keep_unused=True)

    def run(in_maps):
        per_core = [[np.asarray(m[n]) for n in in_names] for m in in_maps]
        concat_in = [
            np.concatenate([per_core[c][i] for c in range(n_cores)], axis=0)
            for i in range(n_params)]
        concat_zeros = [
            np.zeros((n_cores * z.shape[0], *z.shape[1:]), z.dtype)
            for z in zero_outs]
        outs = jitted(*concat_in, *concat_zeros)
        jax.block_until_ready(outs)
        return [
            {n: np.asarray(outs[i]).reshape(n_cores, *out_avals[i].shape)[c]
             for i, n in enumerate(out_names)}
            for c in range(n_cores)]

    return run


def kernel(**inputs):
    x = np.asarray(inputs["x"])
    h_idx = np.asarray(inputs["h_idx"])
    w_idx = np.asarray(inputs["w_idx"])
    proj_w = np.asarray(inputs["proj_w"])
    proj_b = np.asarray(inputs["proj_b"])

    if "nc" not in _cache:
        _cache["nc"] = _build()
        _cache["run"] = _make_runner(_cache["nc"], NCORES)

    maps = _in_maps(x, h_idx, w_idx, proj_w, proj_b)
    results = _cache["run"](maps)

    out = np.stack([results[c]["out"] for c in range(NCORES)])
    # [core, 6*128, 3*768] -> [core, group, p, u, e] -> patch order
    out = out.reshape(NCORES, NCHUNK // 3, 128, 3, EMBED)
    out = out.transpose(0, 1, 3, 2, 4)
    return out.astype(np.float32).reshape(B, N, EMBED)

